# revision 9
# baseline (speedup 1.0000x reference)
"""Trainium2 Bass kernel for the NeighborEmbedding GNN message-passing layer.

Math (per reference):
    cut  = 0.5*(cos(w*pi/5)+1) * (w < 5)                      [E]
    W    = (edge_feats @ W_dist + b_dist) * cut[:,None]       [E,C]
    x_j  = embed_table[node_z][senders]                       [E,C]
    agg  = segment_sum(x_j * W, receivers, N)                 [N,C]
    out  = concat([node_feats, agg], 1) @ W_comb + b_comb     [N,C]

Strategy: receivers are sorted, so shard by contiguous node (receiver)
ranges -> each core owns a contiguous slice of the edge list and its own
slice of the output; no cross-core collective needed.  Per core, edges
are grouped into 128-node output windows.  Each 128-edge tile runs three
PE matmuls (all bf16 -> 1 cyc/row streaming + FWL weight loads):
  mm1: Wt  = efT_tile.T @ W_aug      (K=65: 64 RBF rows + ones row = bias)
  mm2: x_j = ohT_tile.T @ embed      (K=100 one-hot of node_z[senders])
  mm3: agg += S_cut.T @ msgs         (segment-sum as matmul, PSUM accum)
with S_cut[e,m] = cut[e] * (recv_rel[e] == m) built on the HOST and
DMA'd in as bf16 (no per-tile DVE build), and msgs = x_j * Wt via one
ACT evacuation (wt -> SBUF bf16) + one DVE tensor_tensor.  Window
flush: PE-transpose agg, then out = concat(nf,agg) @ W_comb + b_comb as
five accumulating matmuls from pre-transposed node_feats (host layout)
+ transposed agg.
"""

import math
import os
import sys
from contextlib import ExitStack

for _p in ("/opt/trn_rl_repo", "/root/.axon_site/_ro/trn_rl_repo"):
    if os.path.isdir(_p) and _p not in sys.path:
        sys.path.insert(0, _p)

import numpy as np
import ml_dtypes

import concourse.bass as bass
import concourse.tile as tile
from concourse.tile import add_dep_helper
from concourse import bacc
from concourse import mybir

F32 = mybir.dt.float32
F32R = mybir.dt.float32r
BF16 = mybir.dt.bfloat16
NPBF = ml_dtypes.bfloat16

# Bisection toggles: parts of the flush path that may interact badly with
# FWL (enabled automatically for bf16 weights) on real hardware.
BF16_TRANSPOSE = True   # False -> f32r transposes (v1-proven path)
SKIP_ZERO_BIAS = True    # skip the K=1 bias matmul when b_comb == 0

# Problem constants (hardcoded per spec nn_NeighborEmbedding_36146444763345)
N = 50000
E = 800000
C = 256
RBF = 64
NS = 100  # num species
CUTOFF = 5.0
NCORES = 8
NPC = N // NCORES  # nodes per core = 6250
WIN = 128  # output window (nodes)
NW = (NPC + WIN - 1) // WIN  # 49 windows/core
NPAD = NW * WIN  # padded nodes/core = 6272


# --------------------------------------------------------------------------
# Host-side prep: shard + layout
# --------------------------------------------------------------------------


def host_prep(node_z, node_feats, senders, receivers, edge_weight, edge_feats,
              embed_table, W_dist, b_dist, W_comb, b_comb,
              n=N, e=E, ncores=NCORES):
    npc = n // ncores
    nw = (npc + WIN - 1) // WIN
    npad = nw * WIN

    node_z = np.asarray(node_z).astype(np.int64)
    senders = np.asarray(senders).astype(np.int64)
    receivers = np.asarray(receivers).astype(np.int64)
    node_feats = np.asarray(node_feats, dtype=np.float32)
    edge_weight = np.asarray(edge_weight, dtype=np.float32)
    edge_feats = np.asarray(edge_feats, dtype=np.float32)
    embed_table = np.asarray(embed_table, dtype=np.float32)
    W_dist = np.asarray(W_dist, dtype=np.float32)
    b_dist = np.asarray(b_dist, dtype=np.float32)
    W_comb = np.asarray(W_comb, dtype=np.float32)
    b_comb = np.asarray(b_comb, dtype=np.float32)

    zs = node_z[senders]  # [E] species of sender per edge

    # per-core / per-window edge counts (receivers sorted globally)
    win_starts = []  # per core: array of window start edge idx, len nw+1
    for c in range(ncores):
        base = c * npc
        bnd = base + np.minimum(np.arange(nw + 1) * WIN, npc)
        win_starts.append(np.searchsorted(receivers, bnd))
    cnt = np.stack([np.diff(ws) for ws in win_starts])  # [ncores, nw]
    tiles_c = np.maximum(1, (cnt + 127) // 128).astype(int)  # [ncores, nw]
    # Load balance across the SPMD program: slot k of every core runs that
    # core's k-th largest window (host permutes inputs, unpermutes output),
    # so the shared per-slot tile budget is the max of order statistics
    # instead of positional maxima.
    perm = np.argsort(-tiles_c, axis=1, kind="stable")  # [ncores, nw]
    t_w = np.sort(tiles_c, axis=1)[:, ::-1].max(axis=0)  # [nw] per-slot tiles
    n_tiles = int(t_w.sum())
    L = n_tiles * 128

    # shared constants, packed into one tensor to minimize prologue DMAs
    # pack_r: waug 256 | emb 256 | bcomb 256 | ones 128 | ident 128 |
    #         wc 4*256  -> [128, 2048] bf16
    pack_r = np.zeros((128, 2048), np.float32)
    pack_r[:RBF, 0:C] = W_dist
    pack_r[RBF, 0:C] = b_dist
    pack_r[:NS, C:2 * C] = embed_table
    pack_r[0, 2 * C:3 * C] = b_comb
    pack_r[0, 3 * C:3 * C + 128] = 1.0
    pack_r[:, 3 * C + 128:4 * C] = np.eye(128, dtype=np.float32)
    for k in range(4):
        pack_r[:, 1024 + k * C:1024 + (k + 1) * C] = W_comb[k * 128:(k + 1) * 128]
    pack_r = pack_r.astype(NPBF)
    # f32: identity (f32 transpose path) | ones row | b_comb row
    pack_f = np.zeros((128, 256 + C), np.float32)
    pack_f[:, 0:128] = np.eye(128, dtype=np.float32)
    pack_f[0, 128:256] = 1.0
    pack_f[0, 256:256 + C] = b_comb

    # cosine cutoff, exact in f64 then cast
    cutv = 0.5 * (np.cos(edge_weight.astype(np.float64) * math.pi / CUTOFF) + 1.0)
    cutv = (cutv * (edge_weight < CUTOFF)).astype(np.float32)

    in_maps = []
    for c in range(ncores):
        ws = win_starts[c]
        order = np.full(L, -1, dtype=np.int64)
        wbase = np.empty(L, dtype=np.int64)  # window node base per slot
        pos = 0
        for k in range(nw):
            w = int(perm[c, k])
            ne = ws[w + 1] - ws[w]
            order[pos:pos + ne] = np.arange(ws[w], ws[w + 1])
            wbase[pos:pos + t_w[k] * 128] = w * WIN + c * npc
            pos += t_w[k] * 128
        valid = order >= 0
        oc = np.where(valid, order, 0)

        eft = np.empty((RBF + 1, L), np.float32)
        eft[:RBF] = edge_feats[oc].T
        eft[RBF] = 1.0
        eft[:, ~valid] = 0.0

        oh = np.zeros((NS, L), NPBF)
        oh[zs[oc][valid], np.nonzero(valid)[0]] = 1.0

        # S_cut [128, L]: per tile t, block [:, t*128:(t+1)*128] is
        # [e_slot, m] with S[e, m] = cut[e] * (recv_rel[e] == m)
        rr = np.where(valid, receivers[oc] - wbase, -1)
        s_arr = np.zeros((n_tiles, 128, 128), np.float32)
        slot = np.arange(L)
        vs = slot[valid]
        s_arr[vs // 128, vs % 128, rr[valid]] = cutv[oc[valid]]
        s_cut = np.ascontiguousarray(
            s_arr.transpose(1, 0, 2).reshape(128, L)).astype(NPBF)

        nf_pad = np.zeros((npad, C), np.float32)
        nf_pad[:npc] = node_feats[c * npc:(c + 1) * npc]
        # permute node windows into slot order
        nf_slot = nf_pad.reshape(nw, WIN, C)[perm[c]].reshape(npad, C)
        nft = nf_slot.T  # [C, npad]
        nft2 = np.ascontiguousarray(
            np.concatenate([nft[0:128], nft[128:256]], axis=1)).astype(NPBF)

        in_maps.append({
            "eft": eft.astype(NPBF),
            "oh": oh,
            "scut": s_cut,
            "nft": nft2,
            "packr": pack_r,
            "packf": pack_f,
        })

    meta = dict(t_w=t_w, n_tiles=n_tiles, nw=nw, npad=npad, npc=npc,
                ncores=ncores, has_bias=bool(np.any(b_comb)), perm=perm)
    return in_maps, meta


def assemble_output(res, meta):
    """Gather per-core slot-ordered outputs back to node order."""
    npc, nw, perm = meta["npc"], meta["nw"], meta["perm"]
    outs = []
    for c in range(meta["ncores"]):
        o = np.asarray(res.results[c]["out"])
        o_node = np.empty_like(o)
        o_node.reshape(nw, WIN, C)[perm[c]] = o.reshape(nw, WIN, C)
        outs.append(o_node[:npc])
    return np.concatenate(outs, axis=0).astype(np.float32)


# --------------------------------------------------------------------------
# Device program (SPMD: one program, per-core data)
# --------------------------------------------------------------------------


def build_program(meta):
    t_w = meta["t_w"]
    n_tiles = meta["n_tiles"]
    nw = meta["nw"]
    npad = meta["npad"]
    L = n_tiles * 128

    nc = bacc.Bacc("TRN2", target_bir_lowering=False, debug=False)

    eft_d = nc.dram_tensor("eft", [RBF + 1, L], BF16, kind="ExternalInput")
    oh_d = nc.dram_tensor("oh", [NS, L], BF16, kind="ExternalInput")
    scut_d = nc.dram_tensor("scut", [128, L], BF16, kind="ExternalInput")
    nft_d = nc.dram_tensor("nft", [128, 2 * npad], BF16, kind="ExternalInput")
    packr_d = nc.dram_tensor("packr", [128, 2048], BF16, kind="ExternalInput")
    packf_d = nc.dram_tensor("packf", [128, 256 + C], F32, kind="ExternalInput")

    out_d = nc.dram_tensor("out", [npad, C], F32, kind="ExternalOutput")

    with tile.TileContext(nc) as tc, ExitStack() as ctx:
        consts = ctx.enter_context(tc.tile_pool(name="consts", bufs=1))
        eft_p = ctx.enter_context(tc.tile_pool(name="eftp", bufs=3))
        oh_p = ctx.enter_context(tc.tile_pool(name="ohp", bufs=3))
        s_p = ctx.enter_context(tc.tile_pool(name="sp", bufs=3))
        msgs_p = ctx.enter_context(tc.tile_pool(name="msgsp", bufs=4))
        wtb_p = ctx.enter_context(tc.tile_pool(name="wtbp", bufs=2))
        flush_p = ctx.enter_context(tc.tile_pool(name="flushp", bufs=2))
        ps_wt = ctx.enter_context(tc.tile_pool(name="pswt", bufs=2, space="PSUM"))
        ps_xj = ctx.enter_context(tc.tile_pool(name="psxj", bufs=2, space="PSUM"))
        ps_agg = ctx.enter_context(tc.tile_pool(name="psagg", bufs=2, space="PSUM"))
        ps_tr = ctx.enter_context(tc.tile_pool(name="pstr", bufs=1, space="PSUM"))
        ps_out = ctx.enter_context(tc.tile_pool(name="psout", bufs=1, space="PSUM"))

        # ---- constants ----
        packr_sb = consts.tile([128, 2048], BF16)
        nc.sync.dma_start(packr_sb[:], packr_d[:, :])
        nft_sb = consts.tile([128, 2 * npad], BF16)
        waug_sb = packr_sb[:RBF + 1, 0:C]
        emb_sb = packr_sb[:NS, C:2 * C]
        ident = packr_sb[:, 3 * C + 128:4 * C]

        has_bias = meta.get("has_bias", True) or not SKIP_ZERO_BIAS
        TRDT = BF16 if BF16_TRANSPOSE else F32
        packf_sb = None
        if not BF16_TRANSPOSE or has_bias:
            packf_sb = consts.tile([128, 256 + C], F32)
            nc.sync.dma_start(packf_sb[:], packf_d[:, :])
        identf = packf_sb[:, 0:128] if packf_sb is not None else None
        onesf_sb = packf_sb[:1, 128:256] if packf_sb is not None else None
        bcombf_sb = packf_sb[:1, 256:256 + C] if packf_sb is not None else None

        # ---- main loop ----
        # Tiles are processed in groups of 2: mm1/mm2 per tile land in
        # [128,512] PSUM banks; one ACT copy + one DVE tensor_tensor then
        # cover the whole group, halving per-instruction overhead.
        t0 = 0
        for w in range(nw):
            tw = int(t_w[w])
            lw = tw * 128
            c0 = t0 * 128

            eft_w = eft_p.tile([RBF + 1, lw], BF16, tag="eft")
            nc.sync.dma_start(eft_w[:], eft_d[:, c0:c0 + lw])
            oh_w = oh_p.tile([NS, lw], BF16, tag="oh")
            nc.sync.dma_start(oh_w[:], oh_d[:, c0:c0 + lw])
            s_w = s_p.tile([128, lw], BF16, tag="s")
            nc.sync.dma_start(s_w[:], scut_d[:, c0:c0 + lw])
            if w == 0:
                # constants needed only from the first flush; issuing after
                # the window-0 input DMAs lets compute start sooner
                nc.sync.dma_start(nft_sb[:], nft_d[:, :])

            wtb_w = wtb_p.tile([128, tw * C], BF16, tag="wtb",
                               padded_shape=[128, 18 * C])

            agg_ps = ps_agg.tile([128, C], F32, tag="agg")
            for g in range(0, tw, 2):
                gn = min(2, tw - g)  # tiles in this group
                wt_ps = ps_wt.tile([128, 2 * C], F32, tag="wt")
                xj_ps = ps_xj.tile([128, 2 * C], F32, tag="xj")
                mm2_last = None
                for jj in range(gn):
                    j = g + jj
                    sl = slice(j * 128, (j + 1) * 128)
                    nc.tensor.matmul(wt_ps[:, jj * C:(jj + 1) * C],
                                     eft_w[:, sl], waug_sb[:],
                                     start=True, stop=True)
                    mm2_last = nc.tensor.matmul(xj_ps[:, jj * C:(jj + 1) * C],
                                                oh_w[:, sl], emb_sb[:],
                                                start=True, stop=True)

                wtb_sl = wtb_w[:, g * C:(g + gn) * C]
                cp = nc.scalar.copy(wtb_sl, wt_ps[:, 0:gn * C])
                # ACT waits through mm2 too (same PE queue, later count), so
                # the TT's dependency on xj_ps is covered transitively
                add_dep_helper(cp.ins, mm2_last.ins, sync=True,
                               reason="wtb-after-mm2")

                msgs_t = msgs_p.tile([128, 2 * C], BF16, tag="msgs")
                nc.vector.tensor_tensor(out=msgs_t[:, 0:gn * C],
                                        in0=xj_ps[:, 0:gn * C],
                                        in1=wtb_sl,
                                        op=mybir.AluOpType.mult)

                for jj in range(gn):
                    j = g + jj
                    sl = slice(j * 128, (j + 1) * 128)
                    nc.tensor.matmul(agg_ps[:], s_w[:, sl],
                                     msgs_t[:, jj * C:(jj + 1) * C],
                                     start=(j == 0), stop=(j == tw - 1))

            # ---- window flush ----
            agg_sb = flush_p.tile([128, C], TRDT, tag="aggsb")
            nc.scalar.copy(agg_sb[:], agg_ps[:])
            tr_ps = ps_tr.tile([128, C], TRDT, tag="tr")
            tr_id = ident[:] if BF16_TRANSPOSE else identf
            nc.tensor.transpose(tr_ps[:, 0:128], agg_sb[:, 0:128], tr_id)
            nc.tensor.transpose(tr_ps[:, 128:256], agg_sb[:, 128:256], tr_id)
            aggt_sb = flush_p.tile([128, C], BF16, tag="aggtsb")
            nc.vector.tensor_copy(aggt_sb[:], tr_ps[:])

            out_ps = ps_out.tile([128, C], F32, tag="outp")
            for k in range(2):
                nfs = slice(k * npad + w * 128, k * npad + w * 128 + 128)
                nc.tensor.matmul(
                    out_ps[:], nft_sb[:, nfs],
                    packr_sb[:, 1024 + k * C:1024 + (k + 1) * C],
                    start=(k == 0), stop=False)
            for k in range(2):
                nc.tensor.matmul(
                    out_ps[:], aggt_sb[:, k * 128:(k + 1) * 128],
                    packr_sb[:, 1024 + (2 + k) * C:1024 + (3 + k) * C],
                    start=False, stop=(k == 1 and not has_bias))
            if has_bias:
                nc.tensor.matmul(out_ps[:], onesf_sb[:], bcombf_sb[:],
                                 start=False, stop=True)

            out_sb = flush_p.tile([128, C], F32, tag="outsb")
            nc.scalar.copy(out_sb[:], out_ps[:])
            nc.sync.dma_start(out_d[w * 128:(w + 1) * 128, :], out_sb[:])

            t0 += tw

    nc.compile()
    return nc


# --------------------------------------------------------------------------
# Entry point
# --------------------------------------------------------------------------


def kernel(**inputs) -> np.ndarray:
    from concourse.bass_utils import run_bass_kernel_spmd

    in_maps, meta = host_prep(**inputs)
    nc = build_program(meta)
    res = run_bass_kernel_spmd(nc, in_maps, list(range(meta["ncores"])))
    return assemble_output(res, meta)


# revision 10
# speedup vs baseline: 1.4206x; 1.4206x over previous
"""Trainium2 Bass kernel for the NeighborEmbedding GNN message-passing layer.

Math (per reference):
    cut  = 0.5*(cos(w*pi/5)+1) * (w < 5)                      [E]
    W    = (edge_feats @ W_dist + b_dist) * cut[:,None]       [E,C]
    x_j  = embed_table[node_z][senders]                       [E,C]
    agg  = segment_sum(x_j * W, receivers, N)                 [N,C]
    out  = concat([node_feats, agg], 1) @ W_comb + b_comb     [N,C]

Strategy: receivers are sorted, so shard by contiguous node (receiver)
ranges -> each core owns a contiguous slice of the edge list and its own
slice of the output; no cross-core collective needed.  Per core, edges
are grouped into 128-node output windows.  Each 128-edge tile runs three
PE matmuls (all bf16 -> 1 cyc/row streaming + FWL weight loads):
  mm1: Wt  = efT_tile.T @ W_aug      (K=65: 64 RBF rows + ones row = bias)
  mm2: x_j = ohT_tile.T @ embed      (K=100 one-hot of node_z[senders])
  mm3: agg += S_cut.T @ msgs         (segment-sum as matmul, PSUM accum)
with S_cut[e,m] = cut[e] * (recv_rel[e] == m) built on the HOST and
DMA'd in as bf16 (no per-tile DVE build), and msgs = x_j * Wt via one
ACT evacuation (wt -> SBUF bf16) + one DVE tensor_tensor.  Window
flush: PE-transpose agg, then out = concat(nf,agg) @ W_comb + b_comb as
five accumulating matmuls from pre-transposed node_feats (host layout)
+ transposed agg.
"""

import math
import os
import sys
from contextlib import ExitStack

for _p in ("/opt/trn_rl_repo", "/root/.axon_site/_ro/trn_rl_repo"):
    if os.path.isdir(_p) and _p not in sys.path:
        sys.path.insert(0, _p)

import numpy as np
import ml_dtypes

import concourse.bass as bass
import concourse.tile as tile
from concourse import bacc
from concourse import mybir

F32 = mybir.dt.float32
F32R = mybir.dt.float32r
BF16 = mybir.dt.bfloat16
NPBF = ml_dtypes.bfloat16

# Bisection toggles: parts of the flush path that may interact badly with
# FWL (enabled automatically for bf16 weights) on real hardware.
BF16_TRANSPOSE = True   # False -> f32r transposes (v1-proven path)
SKIP_ZERO_BIAS = True    # skip the K=1 bias matmul when b_comb == 0

# Problem constants (hardcoded per spec nn_NeighborEmbedding_36146444763345)
N = 50000
E = 800000
C = 256
RBF = 64
NS = 100  # num species
CUTOFF = 5.0
NCORES = 8
NPC = N // NCORES  # nodes per core = 6250
WIN = 128  # output window (nodes)
NW = (NPC + WIN - 1) // WIN  # 49 windows/core
NPAD = NW * WIN  # padded nodes/core = 6272


# --------------------------------------------------------------------------
# Host-side prep: shard + layout
# --------------------------------------------------------------------------


def host_prep(node_z, node_feats, senders, receivers, edge_weight, edge_feats,
              embed_table, W_dist, b_dist, W_comb, b_comb,
              n=N, e=E, ncores=NCORES):
    npc = n // ncores
    nw = (npc + WIN - 1) // WIN
    npad = nw * WIN

    node_z = np.asarray(node_z).astype(np.int64)
    senders = np.asarray(senders).astype(np.int64)
    receivers = np.asarray(receivers).astype(np.int64)
    node_feats = np.asarray(node_feats, dtype=np.float32)
    edge_weight = np.asarray(edge_weight, dtype=np.float32)
    edge_feats = np.asarray(edge_feats, dtype=np.float32)
    embed_table = np.asarray(embed_table, dtype=np.float32)
    W_dist = np.asarray(W_dist, dtype=np.float32)
    b_dist = np.asarray(b_dist, dtype=np.float32)
    W_comb = np.asarray(W_comb, dtype=np.float32)
    b_comb = np.asarray(b_comb, dtype=np.float32)

    zs = node_z[senders]  # [E] species of sender per edge

    # per-core / per-window edge counts (receivers sorted globally)
    win_starts = []  # per core: array of window start edge idx, len nw+1
    for c in range(ncores):
        base = c * npc
        bnd = base + np.minimum(np.arange(nw + 1) * WIN, npc)
        win_starts.append(np.searchsorted(receivers, bnd))
    cnt = np.stack([np.diff(ws) for ws in win_starts])  # [ncores, nw]
    tiles_c = np.maximum(1, (cnt + 127) // 128).astype(int)  # [ncores, nw]
    # Load balance across the SPMD program: slot k of every core runs that
    # core's k-th largest window (host permutes inputs, unpermutes output),
    # so the shared per-slot tile budget is the max of order statistics
    # instead of positional maxima.
    perm = np.argsort(-tiles_c, axis=1, kind="stable")  # [ncores, nw]
    t_w = np.sort(tiles_c, axis=1)[:, ::-1].max(axis=0)  # [nw] per-slot tiles
    n_tiles = int(t_w.sum())
    L = n_tiles * 128

    # shared constants, packed into one tensor to minimize prologue DMAs
    # pack_r: waug 256 | emb 256 | bcomb 256 | ones 128 | ident 128 |
    #         wc 4*256  -> [128, 2048] bf16
    pack_r = np.zeros((128, 2048), np.float32)
    pack_r[:RBF, 0:C] = W_dist
    pack_r[RBF, 0:C] = b_dist
    pack_r[:NS, C:2 * C] = embed_table
    pack_r[0, 2 * C:3 * C] = b_comb
    pack_r[0, 3 * C:3 * C + 128] = 1.0
    pack_r[:, 3 * C + 128:4 * C] = np.eye(128, dtype=np.float32)
    for k in range(4):
        pack_r[:, 1024 + k * C:1024 + (k + 1) * C] = W_comb[k * 128:(k + 1) * 128]
    pack_r = pack_r.astype(NPBF)
    # f32: identity (f32 transpose path) | ones row | b_comb row
    pack_f = np.zeros((128, 256 + C), np.float32)
    pack_f[:, 0:128] = np.eye(128, dtype=np.float32)
    pack_f[0, 128:256] = 1.0
    pack_f[0, 256:256 + C] = b_comb

    # cosine cutoff, exact in f64 then cast
    cutv = 0.5 * (np.cos(edge_weight.astype(np.float64) * math.pi / CUTOFF) + 1.0)
    cutv = (cutv * (edge_weight < CUTOFF)).astype(np.float32)

    in_maps = []
    for c in range(ncores):
        ws = win_starts[c]
        order = np.full(L, -1, dtype=np.int64)
        wbase = np.empty(L, dtype=np.int64)  # window node base per slot
        pos = 0
        for k in range(nw):
            w = int(perm[c, k])
            ne = ws[w + 1] - ws[w]
            order[pos:pos + ne] = np.arange(ws[w], ws[w + 1])
            wbase[pos:pos + t_w[k] * 128] = w * WIN + c * npc
            pos += t_w[k] * 128
        valid = order >= 0
        oc = np.where(valid, order, 0)

        eft = np.empty((RBF + 1, L), np.float32)
        eft[:RBF] = edge_feats[oc].T
        eft[RBF] = 1.0
        eft[:, ~valid] = 0.0

        oh = np.zeros((NS, L), NPBF)
        oh[zs[oc][valid], np.nonzero(valid)[0]] = 1.0

        # S_cut [128, L]: per tile t, block [:, t*128:(t+1)*128] is
        # [e_slot, m] with S[e, m] = cut[e] * (recv_rel[e] == m)
        rr = np.where(valid, receivers[oc] - wbase, -1)
        s_arr = np.zeros((n_tiles, 128, 128), np.float32)
        slot = np.arange(L)
        vs = slot[valid]
        s_arr[vs // 128, vs % 128, rr[valid]] = cutv[oc[valid]]
        s_cut = np.ascontiguousarray(
            s_arr.transpose(1, 0, 2).reshape(128, L)).astype(NPBF)

        nf_pad = np.zeros((npad, C), np.float32)
        nf_pad[:npc] = node_feats[c * npc:(c + 1) * npc]
        # permute node windows into slot order
        nf_slot = nf_pad.reshape(nw, WIN, C)[perm[c]].reshape(npad, C)
        nft = nf_slot.T  # [C, npad]
        nft2 = np.ascontiguousarray(
            np.concatenate([nft[0:128], nft[128:256]], axis=1)).astype(NPBF)

        in_maps.append({
            "eft": eft.astype(NPBF),
            "oh": oh,
            "scut": s_cut,
            "nft": nft2,
            "packr": pack_r,
            "packf": pack_f,
        })

    meta = dict(t_w=t_w, n_tiles=n_tiles, nw=nw, npad=npad, npc=npc,
                ncores=ncores, has_bias=bool(np.any(b_comb)), perm=perm)
    return in_maps, meta


def assemble_output(res, meta):
    """Gather per-core slot-ordered outputs back to node order."""
    npc, nw, perm = meta["npc"], meta["nw"], meta["perm"]
    outs = []
    for c in range(meta["ncores"]):
        o = np.asarray(res.results[c]["out"])
        o_node = np.empty_like(o)
        o_node.reshape(nw, WIN, C)[perm[c]] = o.reshape(nw, WIN, C)
        outs.append(o_node[:npc])
    return np.concatenate(outs, axis=0).astype(np.float32)


# --------------------------------------------------------------------------
# Device program (SPMD: one program, per-core data)
# --------------------------------------------------------------------------


def build_program(meta):
    t_w = meta["t_w"]
    n_tiles = meta["n_tiles"]
    nw = meta["nw"]
    npad = meta["npad"]
    L = n_tiles * 128

    nc = bacc.Bacc("TRN2", target_bir_lowering=False, debug=False)

    eft_d = nc.dram_tensor("eft", [RBF + 1, L], BF16, kind="ExternalInput")
    oh_d = nc.dram_tensor("oh", [NS, L], BF16, kind="ExternalInput")
    scut_d = nc.dram_tensor("scut", [128, L], BF16, kind="ExternalInput")
    nft_d = nc.dram_tensor("nft", [128, 2 * npad], BF16, kind="ExternalInput")
    packr_d = nc.dram_tensor("packr", [128, 2048], BF16, kind="ExternalInput")
    packf_d = nc.dram_tensor("packf", [128, 256 + C], F32, kind="ExternalInput")

    out_d = nc.dram_tensor("out", [npad, C], F32, kind="ExternalOutput")

    with tile.TileContext(nc) as tc, ExitStack() as ctx:
        consts = ctx.enter_context(tc.tile_pool(name="consts", bufs=1))
        eft_p = ctx.enter_context(tc.tile_pool(name="eftp", bufs=3))
        oh_p = ctx.enter_context(tc.tile_pool(name="ohp", bufs=3))
        s_p = ctx.enter_context(tc.tile_pool(name="sp", bufs=3))
        msgs_p = ctx.enter_context(tc.tile_pool(name="msgsp", bufs=4))
        wtb_p = ctx.enter_context(tc.tile_pool(name="wtbp", bufs=2))
        flush_p = ctx.enter_context(tc.tile_pool(name="flushp", bufs=2))
        ps_wt = ctx.enter_context(tc.tile_pool(name="pswt", bufs=2, space="PSUM"))
        ps_xj = ctx.enter_context(tc.tile_pool(name="psxj", bufs=2, space="PSUM"))
        ps_agg = ctx.enter_context(tc.tile_pool(name="psagg", bufs=2, space="PSUM"))
        ps_tr = ctx.enter_context(tc.tile_pool(name="pstr", bufs=1, space="PSUM"))
        ps_out = ctx.enter_context(tc.tile_pool(name="psout", bufs=1, space="PSUM"))

        # ---- constants ----
        packr_sb = consts.tile([128, 2048], BF16)
        nc.sync.dma_start(packr_sb[:], packr_d[:, :])
        nft_sb = consts.tile([128, 2 * npad], BF16)
        waug_sb = packr_sb[:RBF + 1, 0:C]
        emb_sb = packr_sb[:NS, C:2 * C]
        ident = packr_sb[:, 3 * C + 128:4 * C]

        has_bias = meta.get("has_bias", True) or not SKIP_ZERO_BIAS
        TRDT = BF16 if BF16_TRANSPOSE else F32
        packf_sb = None
        if not BF16_TRANSPOSE or has_bias:
            packf_sb = consts.tile([128, 256 + C], F32)
            nc.sync.dma_start(packf_sb[:], packf_d[:, :])
        identf = packf_sb[:, 0:128] if packf_sb is not None else None
        onesf_sb = packf_sb[:1, 128:256] if packf_sb is not None else None
        bcombf_sb = packf_sb[:1, 256:256 + C] if packf_sb is not None else None

        # ---- main loop ----
        # Tiles are processed in groups of 2: mm1/mm2 per tile land in
        # [128,512] PSUM banks; one ACT copy + one DVE tensor_tensor then
        # cover the whole group, halving per-instruction overhead.
        t0 = 0
        for w in range(nw):
            tw = int(t_w[w])
            lw = tw * 128
            c0 = t0 * 128

            eft_w = eft_p.tile([RBF + 1, lw], BF16, tag="eft")
            nc.sync.dma_start(eft_w[:], eft_d[:, c0:c0 + lw])
            oh_w = oh_p.tile([NS, lw], BF16, tag="oh")
            nc.sync.dma_start(oh_w[:], oh_d[:, c0:c0 + lw])
            s_w = s_p.tile([128, lw], BF16, tag="s")
            nc.sync.dma_start(s_w[:], scut_d[:, c0:c0 + lw])
            if w == 0:
                # constants needed only from the first flush; issuing after
                # the window-0 input DMAs lets compute start sooner
                nc.sync.dma_start(nft_sb[:], nft_d[:, :])

            wtb_w = wtb_p.tile([128, tw * C], BF16, tag="wtb",
                               padded_shape=[128, 18 * C])

            agg_ps = ps_agg.tile([128, C], F32, tag="agg")
            for g in range(0, tw, 2):
                gn = min(2, tw - g)  # tiles in this group
                wt_ps = ps_wt.tile([128, 2 * C], F32, tag="wt")
                xj_ps = ps_xj.tile([128, 2 * C], F32, tag="xj")
                for jj in range(gn):
                    j = g + jj
                    sl = slice(j * 128, (j + 1) * 128)
                    nc.tensor.matmul(wt_ps[:, jj * C:(jj + 1) * C],
                                     eft_w[:, sl], waug_sb[:],
                                     start=True, stop=True)
                    nc.tensor.matmul(xj_ps[:, jj * C:(jj + 1) * C],
                                     oh_w[:, sl], emb_sb[:],
                                     start=True, stop=True)

                wtb_sl = wtb_w[:, g * C:(g + gn) * C]
                nc.scalar.copy(wtb_sl, wt_ps[:, 0:gn * C])

                msgs_t = msgs_p.tile([128, 2 * C], BF16, tag="msgs")
                nc.vector.tensor_tensor(out=msgs_t[:, 0:gn * C],
                                        in0=xj_ps[:, 0:gn * C],
                                        in1=wtb_sl,
                                        op=mybir.AluOpType.mult)

                for jj in range(gn):
                    j = g + jj
                    sl = slice(j * 128, (j + 1) * 128)
                    nc.tensor.matmul(agg_ps[:], s_w[:, sl],
                                     msgs_t[:, jj * C:(jj + 1) * C],
                                     start=(j == 0), stop=(j == tw - 1))

            # ---- window flush ----
            agg_sb = flush_p.tile([128, C], TRDT, tag="aggsb")
            nc.scalar.copy(agg_sb[:], agg_ps[:])
            tr_ps = ps_tr.tile([128, C], TRDT, tag="tr")
            tr_id = ident[:] if BF16_TRANSPOSE else identf
            nc.tensor.transpose(tr_ps[:, 0:128], agg_sb[:, 0:128], tr_id)
            nc.tensor.transpose(tr_ps[:, 128:256], agg_sb[:, 128:256], tr_id)
            aggt_sb = flush_p.tile([128, C], BF16, tag="aggtsb")
            nc.vector.tensor_copy(aggt_sb[:], tr_ps[:])

            out_ps = ps_out.tile([128, C], F32, tag="outp")
            for k in range(2):
                nfs = slice(k * npad + w * 128, k * npad + w * 128 + 128)
                nc.tensor.matmul(
                    out_ps[:], nft_sb[:, nfs],
                    packr_sb[:, 1024 + k * C:1024 + (k + 1) * C],
                    start=(k == 0), stop=False)
            for k in range(2):
                nc.tensor.matmul(
                    out_ps[:], aggt_sb[:, k * 128:(k + 1) * 128],
                    packr_sb[:, 1024 + (2 + k) * C:1024 + (3 + k) * C],
                    start=False, stop=(k == 1 and not has_bias))
            if has_bias:
                nc.tensor.matmul(out_ps[:], onesf_sb[:], bcombf_sb[:],
                                 start=False, stop=True)

            out_sb = flush_p.tile([128, C], F32, tag="outsb")
            nc.scalar.copy(out_sb[:], out_ps[:])
            nc.sync.dma_start(out_d[w * 128:(w + 1) * 128, :], out_sb[:])

            t0 += tw

    nc.compile()
    return nc


# --------------------------------------------------------------------------
# Entry point
# --------------------------------------------------------------------------


def kernel(**inputs) -> np.ndarray:
    from concourse.bass_utils import run_bass_kernel_spmd

    in_maps, meta = host_prep(**inputs)
    nc = build_program(meta)
    res = run_bass_kernel_spmd(nc, in_maps, list(range(meta["ncores"])))
    return assemble_output(res, meta)
